# revision 1
# baseline (speedup 1.0000x reference)
"""BCMGOOLSTM on 8 TRN2 NeuronCores — data-parallel over batch, V12.

Strategy (hardcoded for T=1500, B=16, D=512, L=P=512, G=2048, 8 cores):
  - Shard batch: core i handles b in {2i, 2i+1} (B_loc=2).
  - Host prep: reconstruct block-circulant weights from the index tensors,
    fuse the output projection into the recurrence:
        u_t = h_{t-1} @ Wc + (x_t @ WihT + bias),  Wc = wym_w.T @ W_hh.T
    Gates stay in natural [f, i, o, c] column order.
  - Device phases per core:
      1) w_out = xT.T @ WihT + bias  (bf16 matmul, PSUM->SBUF->DRAM bounce)
      2) sequential LSTM scan, 1500 steps, fully unrolled. Per step the four
         gates' matmuls run as CONCURRENT PE column-tiles (tile_position
         (0,32j)): gate f -> PSUM partitions 0-1, i -> 32-33, o -> 64-65,
         c~ -> 96-97. w_out is injected via col-tiled K=2 identity matmuls.
         One sigmoid activation covers f,i,o (partitions 0:66); one tanh
         covers c~ (96:98). PE transposes (row-tiled at 32g) bring gates to
         [L-on-partitions]; cell update on VectorE; column-halves are
         pipelined across steps (next step's k0/k1 matmul rounds are emitted
         between this step's h0 and h1 transpose groups).
      3) ysT = wymT.T @ h-history matmul over the whole scan -> f32 output.
  - Host post: ys[t, 2i+b, p] = ysT_i[p, 2t+b].

This file is self-contained (includes the walrus single-sync-wait workaround).
"""

import numpy as np
import ml_dtypes

# ---------------------------------------------------------------------------
# Problem constants (hardcoded per spec)
# ---------------------------------------------------------------------------
T = 1500
B, D = 16, 512
L = 512
P_DIM = 512
G = 4 * L          # 2048
NCORES = 8
BLOC = B // NCORES  # 2
S_CHUNK = 8         # scan w_out chunk (steps per DMA)

_BUILt = {}


# ---------------------------------------------------------------------------
# Walrus workaround: at most ONE semaphore wait per instruction
# ---------------------------------------------------------------------------
def _apply_tile_patches():
    import concourse.mybir as mybir
    import concourse.tile as tile_mod
    from concourse.vector_clock import ScopedClock

    def _drain_and_barrier(self, tick_clock, wait_clock):
        nc = self.nc
        drain_inst = nc.sync.drain()
        wait_clock.add_sem_waits(
            drain_inst.ins, ScopedClock({None: tick_clock.global_clock})
        )
        nc.all_engine_barrier()
        assert self.sems is not None
        popped = nc._tile_sem_poison_stack.pop()
        assert popped is self._sem_poison
        nc.clear_and_free_semaphores(list(self.sems.allocated().values()))
        nc.all_engine_barrier()

    tile_mod.TileContext._drain_and_barrier = _drain_and_barrier


def _fix_excess_waits(nc, max_waits=1):
    import concourse.mybir as mybir

    counter = 0
    for f in nc.m.functions:
        for blk in f.blocks:
            insts = list(blk.instructions)
            out = []
            changed = False
            for inst in insts:
                si = inst.sync_info
                if si is not None and len(si.on_wait) > max_waits:
                    waits = list(si.on_wait)
                    excess, keep = waits[:-max_waits], waits[-max_waits:]
                    for w in excess:
                        nop = mybir.InstNoOp(
                            name=f"waitspill-{counter}", ins=[], outs=[]
                        )
                        counter += 1
                        nop.engine = inst.engine
                        nop.sync_info = mybir.SyncInfo(on_wait=[w], on_update=[])
                        out.append(nop)
                    inst.sync_info = mybir.SyncInfo(
                        on_wait=keep, on_update=list(si.on_update)
                    )
                    changed = True
                out.append(inst)
            if changed:
                blk.instructions = out
    return counter


# ---------------------------------------------------------------------------
# Device kernel builder
# ---------------------------------------------------------------------------
def _build(t_steps=T):
    import os
    import concourse.bass as bass
    import concourse.mybir as mybir
    from concourse.tile import TileContext, add_dep_helper

    ablate = set(os.environ.get("V12_ABLATE", "").split(","))

    TT = t_steps
    TB = TT * BLOC
    F32 = mybir.dt.float32
    BF16 = mybir.dt.bfloat16
    Sigmoid = mybir.ActivationFunctionType.Sigmoid
    Tanh = mybir.ActivationFunctionType.Tanh

    nc = bass.Bass(trn_type="TRN2")
    xT_d = nc.declare_dram_parameter("xT", [D, TB], F32, isOutput=False)
    wihT_d = nc.declare_dram_parameter("wihT", [D, G], F32, isOutput=False)
    wc_d = nc.declare_dram_parameter("wc", [L, G], F32, isOutput=False)
    wymT_d = nc.declare_dram_parameter("wymT", [L, P_DIM], F32, isOutput=False)
    bias_d = nc.declare_dram_parameter("bias", [1, G], F32, isOutput=False)
    ident_d = nc.declare_dram_parameter("ident", [128, 2], F32, isOutput=False)
    sel_d = nc.declare_dram_parameter("sel", [128, 8], F32, isOutput=False)
    debug = TT <= 16
    if debug:
        dbg_hist_d = nc.declare_dram_parameter(
            "dbg_hist", [128, 4 * (TT + 1) * BLOC], mybir.dt.bfloat16, isOutput=True)
        dbg_sg_d = nc.declare_dram_parameter(
            "dbg_sg", [128, TT * 512], mybir.dt.bfloat16, isOutput=True)
        dbg_u_d = nc.declare_dram_parameter(
            "dbg_u", [128, 4 * 512], mybir.dt.bfloat16, isOutput=True)
    ysT_d = nc.declare_dram_parameter("ysT", [P_DIM, TB], F32, isOutput=True)

    n_mtiles = (TB + 127) // 128

    with TileContext(nc) as tc:
        with tc.tile_pool(name="persist", bufs=1) as pp, \
             tc.tile_pool(name="dram", bufs=1, space="DRAM") as dp:

            # persistent SBUF tensors
            xT_sb = pp.tile([128, 4, TB], BF16)
            wihT_sb = pp.tile([128, 4, G], BF16)
            wc_sb = pp.tile([128, 4, G], BF16)
            wymT_sb = pp.tile([128, 4, P_DIM], BF16)
            bias_sb = pp.tile([1, G], BF16)
            ones_sb = pp.tile([1, 128], BF16)
            ident = pp.tile([128, 2], BF16)
            sel_sb = pp.tile([128, 8], BF16)
            ht_hist = pp.tile([128, 4, TT + 1, BLOC], BF16)

            nc.gpsimd.dma_start(xT_sb[:], xT_d.rearrange("(k p) n -> p k n", p=128))
            nc.gpsimd.dma_start(wihT_sb[:], wihT_d.rearrange("(k p) g -> p k g", p=128))
            nc.gpsimd.dma_start(wc_sb[:], wc_d.rearrange("(k p) g -> p k g", p=128))
            nc.gpsimd.dma_start(wymT_sb[:], wymT_d.rearrange("(k p) g -> p k g", p=128))
            nc.gpsimd.dma_start(bias_sb[:], bias_d[:])
            nc.gpsimd.dma_start(ident[:], ident_d[:])
            nc.gpsimd.dma_start(sel_sb[:], sel_d[:])
            nc.vector.memset(ones_sb[:], 1.0)
            nc.vector.memset(ht_hist[:, :, 0, :], 0.0)

            wout_i = dp.tile([TB, G], BF16)

            # ---------------- phase 1: w_out ----------------
            p1_dmas = []
            with tc.tile_pool(name="p1sb", bufs=3) as p1, \
                 tc.tile_pool(name="p1ps", bufs=3, space="PSUM") as p1p:
                for m in range(n_mtiles):
                    rows = min(128, TB - m * 128)
                    wtile = p1.tile([128, G], BF16, tag="wtile")
                    for nchunk in range(4):
                        pw = p1p.tile([128, 512], F32, tag="pw")
                        for k in range(4):
                            nc.tensor.matmul(
                                pw[:rows],
                                xT_sb[:, k, m * 128 : m * 128 + rows],
                                wihT_sb[:, k, nchunk * 512 : (nchunk + 1) * 512],
                                start=(k == 0),
                                stop=False,
                            )
                        nc.tensor.matmul(
                            pw[:rows],
                            ones_sb[:, :rows],
                            bias_sb[:, nchunk * 512 : (nchunk + 1) * 512],
                            start=False,
                            stop=True,
                        )
                        nc.vector.tensor_copy(
                            wtile[:rows, nchunk * 512 : (nchunk + 1) * 512],
                            pw[:rows],
                        )
                    p1_dmas.append(
                        nc.sync.dma_start(
                            wout_i[m * 128 : m * 128 + rows, :],
                            wtile[:rows],
                        )
                    )

            # ---------------- phase 2: scan (V12, col-tiled) ----------------
            # strips: j=0 f, j=1 i, j=2 o, j=3 c~ at psum partitions 32j.
            # All four gates go through ONE tanh per column-half using
            # sigmoid(x) = (1+tanh(x/2))/2: per-partition scale 0.5 for
            # f/i/o rows, 1.0 for c~ rows. The (1+y)/2 fixup folds into the
            # cell's scalar_tensor_tensor ops with a 2h state convention
            # (host pre-halves Wc and wymT), so no extra elementwise work.
            with tc.tile_pool(name="wop", bufs=2) as wp, \
                 tc.tile_pool(name="sgp", bufs=2) as sgp, \
                 tc.tile_pool(name="cellp", bufs=2) as cp, \
                 tc.tile_pool(name="state", bufs=1) as stp, \
                 tc.tile_pool(name="scanps", bufs=1, space="PSUM") as sps:

                ctT = stp.tile([128, 4, BLOC], F32)  # holds 2*c_t
                nc.vector.memset(ctT[:], 0.0)
                scale_sb = stp.tile([128, 1], F32)
                nc.vector.memset(scale_sb[:], 1.0)
                nc.vector.memset(scale_sb[0:66, :], 0.5)

                # one full PSUM bank per (ping-pong, half) for both the
                # gate accumulators and the transpose outputs: PE writing a
                # bank while ACT/DVE reads the same bank is a fatal PSUM
                # collision, so halves must live in separate banks.
                pub00 = sps.tile([128, 512], F32)
                pub01 = sps.tile([128, 512], F32)
                pub10 = sps.tile([128, 512], F32)
                pub11 = sps.tile([128, 512], F32)
                tpb00 = sps.tile([128, 512], F32)
                tpb01 = sps.tile([128, 512], F32)
                tpb10 = sps.tile([128, 512], F32)
                tpb11 = sps.tile([128, 512], F32)
                pus = [[pub00, pub01], [pub10, pub11]]
                tbanks = [[tpb00, tpb01], [tpb10, tpb11]]
                tviews = [
                    [
                        b[:, 0 : 8 * BLOC].rearrange(
                            "p (k g b) -> p k g b", k=2, g=4, b=BLOC
                        )
                        for b in row
                    ]
                    for row in tbanks
                ]
                for row in pus:
                    for b in row:
                        nc.vector.memset(b[:], 0.0)

                def dma_chunk(cidx):
                    woc = wp.tile([BLOC, S_CHUNK, G], BF16, tag="wo")
                    n_here = min(S_CHUNK, TT - cidx * S_CHUNK)
                    rd = nc.sync.dma_start(
                        woc[:, :n_here, :],
                        wout_i[
                            cidx * S_CHUNK * 2 : (cidx * S_CHUNK + n_here) * 2, :
                        ].rearrange("(t b) g -> b t g", b=2),
                    )
                    # RAW through DRAM isn't tile-tracked: order the chunk
                    # read after the phase-1 write that produced it.
                    m = (cidx * S_CHUNK) // 64
                    m2 = (cidx * S_CHUNK + S_CHUNK - 1) // 64
                    add_dep_helper(rd.ins, p1_dmas[m].ins, reason="wout RAW")
                    if m2 != m and m2 < len(p1_dmas):
                        add_dep_helper(rd.ins, p1_dmas[m2].ins, reason="wout RAW2")
                    return woc

                # MM round emitters. Round (h, r): r=-1 is the K2 w_out
                # injection for half h; r=0..3 are the hT k-chunk rounds.
                # pub = the [128, 512] bank of half h; only cols 0:256 used.
                def emit_round(pub, woc, s, t, h, r):
                    c0 = 256 * h
                    for j in range(4):
                        cols = slice(512 * j + c0, 512 * j + c0 + 256)
                        if r < 0:
                            nc.tensor.matmul(
                                pub[32 * j : 32 * j + 2, 0:256],
                                ident[0:2, :],
                                woc[:, s, cols],
                                start=True,
                                stop=False,
                                tile_position=(0, 32 * j),
                            )
                        else:
                            nc.tensor.matmul(
                                pub[32 * j : 32 * j + 2, 0:256],
                                ht_hist[:, r, t, :],
                                wc_sb[:, r, cols],
                                start=False,
                                stop=(r == 3),
                                tile_position=(0, 32 * j),
                            )

                def emit_acts(pub, sg, h):
                    c = slice(256 * h, 256 * h + 256)
                    nc.scalar.activation(
                        sg[0:98, c], pub[0:98, 0:256], Tanh,
                        scale=scale_sb[0:98, :],
                    )

                def emit_transposes(tpv, sg, h):
                    # gather-transpose all 4 gates of one k-chunk with a
                    # single standard matmul: out[c, (g,b)] =
                    # sum_p sg[p, 128k+c] * sel[p, (g,b)], sel a 0/1
                    # selector of partitions {32g+b}. Avoids row-positioned
                    # PE tiles (which crash when mixed with col-tiles).
                    for k in (2 * h, 2 * h + 1):
                        nc.tensor.matmul(
                            tpv[:, k - 2 * h, :, :],
                            sg[0:98, 128 * k : 128 * (k + 1)],
                            sel_sb[0:98, :],
                            start=True,
                            stop=True,
                        )

                def emit_cell(tpv, sg2, t, h, eng):
                    # gates are y = tanh(u/2) (f/i/o) and c~ = tanh(u);
                    # state ctT = 2*c, ht_hist = 2*h:
                    #   t2 = (yf+1)*ct2_prev        = 4*sig(f)*c_prev
                    #   t1 = (yi+1)*c~              = 2*sig(i)*c~
                    #   ct2 = 0.5*t2 + t1           = 2*c_new
                    #   tanh_ct = tanh(0.5*ct2)     = tanh(c_new)
                    #   ht2 = (yo+1)*tanh_ct        = 2*h_new
                    kk = slice(2 * h, 2 * h + 2)
                    add = mybir.AluOpType.add
                    mult = mybir.AluOpType.mult
                    # stt allows only one PSUM input: bounce c~T to SBUF.
                    cc = cp.tile([128, 2, BLOC], BF16, tag=f"cc{h}")
                    eng.tensor_copy(cc[:], tpv[:, :, 3, :])
                    yf, yi, yo = (tpv[:, :, g, :] for g in (0, 1, 2))
                    t2 = cp.tile([128, 2, BLOC], F32, tag=f"t2{h}")
                    eng.scalar_tensor_tensor(
                        t2[:], yf, 1.0, ctT[:, kk, :], add, mult
                    )
                    t1 = cp.tile([128, 2, BLOC], F32, tag=f"t1{h}")
                    eng.scalar_tensor_tensor(
                        t1[:], yi, 1.0, cc[:], add, mult
                    )
                    eng.scalar_tensor_tensor(
                        ctT[:, kk, :], t2[:], 0.5, t1[:], mult, add
                    )
                    nc.scalar.activation(sg2[:, kk, :], ctT[:, kk, :], Tanh, scale=0.5)
                    eng.scalar_tensor_tensor(
                        ht_hist[:, kk, t + 1, :],
                        yo, 1.0, sg2[:, kk, :], add, mult,
                    )

                # Software pipeline over steps. PE program order per step:
                #   [transp h1(t-1)] [k0,k1 rounds t] [k2,k3 rounds t]
                #   [K2 rounds t+1] [transp h0(t)]
                # ACT order: tanh_ct-h1(t-1), tanh-h0(t), tanh-h1(t),
                #   tanh_ct-h0(t). Cell h0 on DVE, cell h1 on Pool.
                woc = dma_chunk(0)
                woc_next = dma_chunk(1) if TT > S_CHUNK else None
                sgs = [None, None]

                # prologue: K2s of step 0
                emit_round(pus[0][0], woc, 0, 0, 0, -1)
                emit_round(pus[0][1], woc, 0, 0, 1, -1)

                for t in range(TT):
                    puh = pus[t % 2]
                    tvh = tviews[t % 2]
                    s = t % S_CHUNK
                    if s == 0 and t > 0:
                        woc = woc_next
                    if s == S_CHUNK // 2:
                        nxt = t // S_CHUNK + 1
                        if nxt * S_CHUNK < TT:
                            woc_next = dma_chunk(nxt)

                    sg = sgp.tile([128, 512], BF16, tag="sg")
                    sg2 = sgp.tile([128, 4, BLOC], BF16, tag="sg2")
                    sgs[t % 2] = (sg, sg2)

                    # PE: transposes h1 of step t-1, then its cell: these
                    # MUST be emitted before the k2/k3 rounds of step t that
                    # read ht chunks 2,3 (the tile dep tracker is
                    # emission-order based).
                    if t > 0 and "notr" not in ablate and "noact" not in ablate:
                        emit_transposes(tviews[(t - 1) % 2][1], sgs[(t - 1) % 2][0], 1)
                    if t > 0 and not ablate & {"nocell", "notr", "noact"}:
                        emit_cell(
                            tviews[(t - 1) % 2][1], sgs[(t - 1) % 2][1], t - 1, 1,
                            nc.vector,
                        )
                    # PE: k rounds of step t (K2s were emitted last iteration)
                    emit_round(puh[0], woc, s, t, 0, 0)
                    emit_round(puh[0], woc, s, t, 0, 1)
                    emit_round(puh[1], woc, s, t, 1, 0)
                    emit_round(puh[1], woc, s, t, 1, 1)
                    emit_round(puh[0], woc, s, t, 0, 2)
                    emit_round(puh[0], woc, s, t, 0, 3)
                    emit_round(puh[1], woc, s, t, 1, 2)
                    emit_round(puh[1], woc, s, t, 1, 3)
                    # ACT: tanh of both halves of step t
                    if "noact" not in ablate:
                        emit_acts(puh[0], sg, 0)
                        emit_acts(puh[1], sg, 1)
                    if debug:
                        nc.sync.dma_start(
                            dbg_sg_d[:, t * 512 : (t + 1) * 512], sg[:])
                        if t < 4:
                            ub = sgp.tile([128, 512], BF16, tag="dbgu")
                            nc.vector.tensor_copy(ub[:, 0:256], puh[0][:, 0:256])
                            nc.vector.tensor_copy(ub[:, 256:512], puh[1][:, 0:256])
                            nc.sync.dma_start(
                                dbg_u_d[:, t * 512 : (t + 1) * 512], ub[:])
                    # PE: K2 rounds of step t+1
                    if t + 1 < TT:
                        s1 = (t + 1) % S_CHUNK
                        w1 = woc_next if (s1 == 0 and woc_next is not None) else woc
                        emit_round(pus[(t + 1) % 2][0], w1, s1, t + 1, 0, -1)
                        emit_round(pus[(t + 1) % 2][1], w1, s1, t + 1, 1, -1)
                    # PE: transposes h0 of step t, then cell h0 on DVE
                    if not ablate & {"notr", "noact"}:
                        emit_transposes(tvh[0], sg, 0)
                    if not ablate & {"nocell", "notr", "noact"}:
                        emit_cell(tvh[0], sg2, t, 0, nc.vector)

                # drain: last step's h1
                if not ablate & {"notr", "noact"}:
                    emit_transposes(tviews[(TT - 1) % 2][1], sgs[(TT - 1) % 2][0], 1)
                if not ablate & {"nocell", "notr", "noact"}:
                    emit_cell(tviews[(TT - 1) % 2][1], sgs[(TT - 1) % 2][1], TT - 1, 1, nc.vector)
                if debug:
                    nc.sync.dma_start(
                        dbg_hist_d[:],
                        ht_hist[:].rearrange("p k t b -> p (k t b)"))

            # ---------------- phase 3: ys ----------------
            with tc.tile_pool(name="p3sb", bufs=3) as p3, \
                 tc.tile_pool(name="p3ps", bufs=3, space="PSUM") as p3p:
                NT = 512
                n_nt = (TB + NT - 1) // NT
                for m in range(4):
                    for nt in range(n_nt):
                        cols = min(NT, TB - nt * NT)
                        py = p3p.tile([128, NT], F32, tag="py")
                        for k in range(4):
                            nc.tensor.matmul(
                                py[:, :cols],
                                wymT_sb[:, k, m * 128 : (m + 1) * 128],
                                ht_hist[:, k, :, :].rearrange("p t b -> p (t b)")[
                                    :, 2 + nt * NT : 2 + nt * NT + cols
                                ],
                                start=(k == 0),
                                stop=(k == 3),
                            )
                        ytile = p3.tile([128, NT], F32, tag="ytile")
                        nc.vector.tensor_copy(ytile[:, :cols], py[:, :cols])
                        nc.sync.dma_start(
                            ysT_d.rearrange("(mm p) n -> mm p n", p=128)[
                                m, :, nt * NT : nt * NT + cols
                            ],
                            ytile[:, :cols],
                        )

    _fix_excess_waits(nc)
    return nc


def _get_nc(t_steps=T):
    key = t_steps
    if key not in _BUILt:
        _apply_tile_patches()
        _BUILt[key] = _build(t_steps)
    return _BUILt[key]


# ---------------------------------------------------------------------------
# Host entry point
# ---------------------------------------------------------------------------
def _prep_in_maps(x, vector_ih, vector_hh, bias_ih, wym_w, indx_ih, indx_hh,
                  t_steps=T):
    x = np.asarray(x, dtype=np.float32)
    vector_ih = np.asarray(vector_ih, dtype=np.float32)
    vector_hh = np.asarray(vector_hh, dtype=np.float32)
    bias_ih = np.asarray(bias_ih, dtype=np.float32)
    wym_w = np.asarray(wym_w, dtype=np.float32)
    indx_ih = np.asarray(indx_ih)
    indx_hh = np.asarray(indx_hh)

    TB = t_steps * BLOC
    # reconstruct weights (host-side layout prep); gates stay [f, i, o, c].
    # The device stores state as 2*h (sigmoid-via-tanh trick), so Wc and
    # wymT are pre-halved to compensate.
    wihT = vector_ih[indx_ih.reshape(-1).astype(np.int64)].reshape(D, G)
    whh = vector_hh[indx_hh.reshape(-1).astype(np.int64)].reshape(P_DIM, G)
    wc = (wym_w.T.astype(np.float64) @ whh.astype(np.float64)).astype(np.float32)
    wc = wc * 0.5

    bias = np.ascontiguousarray(bias_ih).reshape(1, G)
    wymT = np.ascontiguousarray(wym_w.T) * 0.5
    ident = np.zeros((128, 2), dtype=np.float32)
    for g in range(4):
        ident[32 * g : 32 * g + 2, :] = np.eye(2, dtype=np.float32)
    sel = np.zeros((128, 8), dtype=np.float32)
    for g in range(4):
        for b in range(BLOC):
            sel[32 * g + b, 2 * g + b] = 1.0

    in_maps = []
    for i in range(NCORES):
        x_loc = x[:t_steps, 2 * i : 2 * i + 2, :].reshape(TB, D)
        xT = np.ascontiguousarray(x_loc.T)
        in_maps.append({
            "xT": xT,
            "wihT": np.ascontiguousarray(wihT),
            "wc": np.ascontiguousarray(wc),
            "wymT": wymT,
            "bias": bias,
            "ident": ident,
            "sel": sel,
        })
    return in_maps


def kernel(x, vector_ih, vector_hh, bias_ih, wym_w, indx_ih, indx_hh):
    from concourse.bass_utils import run_bass_kernel_spmd

    in_maps = _prep_in_maps(
        x, vector_ih, vector_hh, bias_ih, wym_w, indx_ih, indx_hh
    )
    nc = _get_nc()
    res = run_bass_kernel_spmd(nc, in_maps, core_ids=list(range(NCORES)))
    globals()["_LAST_RES"] = res

    out = np.empty((T, B, P_DIM), dtype=np.float32)
    for i in range(NCORES):
        ysT = res.results[i]["ysT"]  # [P, TB]
        ys_loc = ysT.T.reshape(T, BLOC, P_DIM)
        out[:, 2 * i : 2 * i + 2, :] = ys_loc
    return out

